# revision 2
# baseline (speedup 1.0000x reference)
"""Trainium2 Bass kernel for a 2-layer LightGCN-style ItemConv.

Math restructure (W right-multiplication commutes with the linear SpMM):
    y1 = (A x) W1^T,  y2 = (A A x) W1^T W2^T,  A = D^-1 (adj + I)
    out = x/3 + n(y1)/3 + n(y2)/3        (n = row L2-normalize)
Row-normalize kills any positive per-row scaling, so all D^-1 factors are
either folded into the host-side gather stream or cancelled.

Device work per layer (per core, 6250 dst rows):
    z_raw^T tile [100,128] = sum over edge-chunks of  G_chunk^T @ S_chunk
where G_chunk = 128 gathered source rows (host pre-gathers into a
sequential stream, uniform chunk layout across all 8 cores so one SPMD
program serves every core and both layers), and S_chunk is a 0/1 one-hot
[128 edges, 32 dst-window] built on-device with a single DVE is_equal.
Then y_raw = z_raw @ Wt on PE, row-normalized and accumulated.

Two launches of the SAME compiled program; the host performs the halo
exchange (re-gather of layer-1 output) between them.
"""

import sys

sys.path.insert(0, "/opt/trn_rl_repo")

import numpy as np

N = 50000
D = 100
NCORES = 8
RPC = N // NCORES            # 6250 dst rows per core
TILES = (RPC + 127) // 128   # 49
PADR = TILES * 128           # 6272
WIN = 32                     # one-hot width / dst window
NWIN = 128 // WIN            # 4
CE = 128                     # edges per chunk
GCH = 32                     # chunks per staged group

_CACHE: dict = {}
LAST_EXEC_NS: list = []      # appended per launch when TRACE
TRACE = False


def _prep_graph(src, dst):
    """Uniform (across cores) chunk layout for the edge list + self loops."""
    loop = np.arange(N, dtype=np.int64)
    src_all = np.concatenate([np.asarray(src, dtype=np.int64), loop])
    dst_all = np.concatenate([np.asarray(dst, dtype=np.int64), loop])
    deg = np.bincount(dst_all, minlength=N).astype(np.float32)
    deg_inv = (1.0 / deg).astype(np.float32)

    order = np.argsort(dst_all, kind="stable")
    s_s = src_all[order]
    d_s = dst_all[order]

    core = d_s // RPC
    dloc = d_s - core * RPC
    tile = dloc // 128
    wloc = (dloc % 128) // WIN
    slot = (core * TILES + tile) * NWIN + wloc      # globally non-decreasing
    nslots = NCORES * TILES * NWIN

    cnt = np.bincount(slot, minlength=nslots).reshape(NCORES, TILES, NWIN)
    nch = -(-cnt // CE)                              # ceil
    nch = nch.max(axis=0)                            # [TILES, NWIN] uniform
    ncht_raw = int(nch.sum())
    ncht = -(-ncht_raw // GCH) * GCH                 # pad to group multiple

    # chunk offset of each (tile, window) slot in the global chunk stream
    off = np.zeros(TILES * NWIN, dtype=np.int64)
    np.cumsum(nch.reshape(-1)[:-1], out=off[1:])
    off = off.reshape(TILES, NWIN)

    # position of each edge inside its slot (edges already sorted by slot)
    first = np.searchsorted(slot, np.arange(nslots), side="left")
    pos_in_slot = np.arange(slot.size, dtype=np.int64) - first[slot]

    g_src = np.zeros((NCORES, ncht * CE), dtype=np.int64)
    rel = np.full((NCORES, ncht * CE), -1.0, dtype=np.float32)
    gpos = off[tile, wloc] * CE + pos_in_slot        # position inside core stream
    for c in range(NCORES):
        m = core == c
        g_src[c, gpos[m]] = s_s[m]
        rel[c, gpos[m]] = (dloc[m] % WIN).astype(np.float32)

    rel_bp = np.ascontiguousarray(
        rel.reshape(NCORES, ncht, CE).transpose(0, 2, 1)
    )                                                # [NCORES, 128, ncht]
    return g_src, rel_bp, nch, ncht, deg_inv


def _gather_stream(mat, g_src):
    """mat [N, D] -> per-core partition-major chunk stream [128, ncht, D]."""
    out = []
    for c in range(NCORES):
        rows = mat[g_src[c]]                         # [ncht*128, D]
        ncht = rows.shape[0] // CE
        out.append(
            np.ascontiguousarray(
                rows.reshape(ncht, CE, D).transpose(1, 0, 2)
            )
        )
    return out


def _build_program(nch, ncht):
    import concourse.bacc as bacc
    import concourse.mybir as mybir
    from concourse import tile

    dt = mybir.dt
    f32 = dt.float32
    nc = bacc.Bacc(
        "TRN2", target_bir_lowering=False, debug=False, num_devices=NCORES
    )

    g_d = nc.dram_tensor("g", [128, ncht, D], f32, kind="ExternalInput")
    rel_d = nc.dram_tensor("rel", [128, ncht], f32, kind="ExternalInput")
    base_d = nc.dram_tensor("base", [PADR, D], f32, kind="ExternalInput")
    wt_d = nc.dram_tensor("wt", [D, D], f32, kind="ExternalInput")
    iota_d = nc.dram_tensor("iota", [128, GCH, WIN], f32, kind="ExternalInput")
    zer_d = nc.dram_tensor("zer", [128, 128], f32, kind="ExternalInput")
    zt_d = nc.dram_tensor("zt", [TILES, D, 128], f32, kind="ExternalOutput")
    out_d = nc.dram_tensor("out", [PADR, D], f32, kind="ExternalOutput")

    ngroups = ncht // GCH

    with tile.TileContext(nc) as tc:
        with (
            tc.tile_pool(name="const", bufs=1) as cp,
            tc.tile_pool(name="gpool", bufs=3) as gp,
            tc.tile_pool(name="spool", bufs=3) as sp,
            tc.tile_pool(name="work", bufs=3) as wp,
            tc.tile_pool(name="small", bufs=4) as smp,
            tc.tile_pool(name="psz", bufs=2, space="PSUM") as pszp,
            tc.tile_pool(name="psy", bufs=2, space="PSUM") as psyp,
        ):
            wt_sb = cp.tile([D, D], f32)
            nc.sync.dma_start(wt_sb[:], wt_d[:])
            iota_sb = cp.tile([128, GCH, WIN], f32)
            nc.sync.dma_start(iota_sb[:], iota_d[:])
            zer_sb = cp.tile([128, 128], f32)
            nc.sync.dma_start(zer_sb[:], zer_d[:])
            rel_sb = cp.tile([128, ncht], f32)
            nc.sync.dma_start(rel_sb[:], rel_d[:])

            state = {"grp": -1, "g": None, "s": None}

            def ensure_group(cg):
                grp = cg // GCH
                if grp == state["grp"]:
                    return
                state["grp"] = grp
                gt = gp.tile([128, GCH, D], f32)
                nc.sync.dma_start(
                    gt[:], g_d[:, grp * GCH : (grp + 1) * GCH, :]
                )
                st = sp.tile([128, GCH, WIN], f32)
                relb = (
                    rel_sb[:, grp * GCH : (grp + 1) * GCH]
                    .unsqueeze(2)
                    .broadcast_to([128, GCH, WIN])
                )
                nc.vector.tensor_tensor(
                    st[:], iota_sb[:], relb, mybir.AluOpType.is_equal
                )
                state["g"] = gt
                state["s"] = st

            cg = 0
            for t in range(TILES):
                pz = pszp.tile([D, 128], f32)
                nc.tensor.matmul(
                    pz[:, :],
                    zer_sb[:, 0:D],
                    zer_sb[:, :],
                    start=True,
                    stop=False,
                )
                ntile = int(nch[t].sum())
                done = 0
                for w in range(NWIN):
                    for _k in range(int(nch[t, w])):
                        ensure_group(cg)
                        ci = cg % GCH
                        done += 1
                        nc.tensor.matmul(
                            pz[:, w * WIN : (w + 1) * WIN],
                            state["g"][:, ci, :],
                            state["s"][:, ci, :],
                            start=False,
                            stop=(done == ntile),
                        )
                        cg += 1

                zt_sb = wp.tile([D, 128], f32)
                nc.vector.tensor_copy(zt_sb[:], pz[:])
                nc.sync.dma_start(zt_d[t], zt_sb[:])

                py = psyp.tile([128, D], f32)
                nc.tensor.matmul(
                    py[:], zt_sb[:], wt_sb[:], start=True, stop=True
                )

                sqs = wp.tile([128, D], f32)
                ssq = smp.tile([128, 1], f32)
                nc.scalar.activation(
                    sqs[:],
                    py[:],
                    mybir.ActivationFunctionType.Square,
                    accum_out=ssq[:],
                )
                nrm = smp.tile([128, 1], f32)
                nc.scalar.activation(
                    nrm[:],
                    ssq[:],
                    mybir.ActivationFunctionType.Sqrt,
                    scale=9.0,
                )
                rsq = smp.tile([128, 1], f32)
                nc.vector.reciprocal(rsq[:], nrm[:])
                bt = wp.tile([128, D], f32)
                nc.sync.dma_start(bt[:], base_d[t * 128 : (t + 1) * 128, :])
                yn = wp.tile([128, D], f32)
                nc.vector.tensor_scalar(
                    yn[:], py[:], rsq[:], None, mybir.AluOpType.mult
                )
                ot = wp.tile([128, D], f32)
                nc.vector.tensor_tensor(
                    ot[:], yn[:], bt[:], mybir.AluOpType.add
                )
                nc.sync.dma_start(out_d[t * 128 : (t + 1) * 128, :], ot[:])

    nc.compile()
    return nc


def _run(nc, in_maps):
    from concourse.bass_utils import run_bass_kernel_spmd

    res = run_bass_kernel_spmd(
        nc, in_maps, list(range(NCORES)), trace=TRACE
    )
    if res.exec_time_ns is not None:
        LAST_EXEC_NS.append(res.exec_time_ns)
    return res.results


def kernel(features, W, src, dst):
    features = np.asarray(features, dtype=np.float32)
    W = np.asarray(W, dtype=np.float32)

    key = (hash(np.asarray(src).tobytes()), hash(np.asarray(dst).tobytes()))
    if key in _CACHE:
        nc, g_src, rel_bp, nch, ncht, deg_inv = _CACHE[key]
    else:
        g_src, rel_bp, nch, ncht, deg_inv = _prep_graph(src, dst)
        nc = _build_program(nch, ncht)
        _CACHE.clear()
        _CACHE[key] = (nc, g_src, rel_bp, nch, ncht, deg_inv)

    iota = np.ascontiguousarray(
        np.broadcast_to(
            np.arange(WIN, dtype=np.float32), (128, GCH, WIN)
        )
    )
    zer = np.zeros((128, 128), dtype=np.float32)

    x3 = features / 3.0
    wt1 = np.ascontiguousarray(W[0].T)
    wt2 = np.ascontiguousarray((W[1] @ W[0]).T)

    # ---- launch 1: z1_raw = M x3 ; out = x3 + n(y1)/3 -------------------
    g1 = _gather_stream(x3, g_src)
    in_maps1 = []
    for c in range(NCORES):
        base = np.zeros((PADR, D), dtype=np.float32)
        base[:RPC] = x3[c * RPC : (c + 1) * RPC]
        in_maps1.append(
            {
                "g": g1[c],
                "rel": rel_bp[c],
                "base": base,
                "wt": wt1,
                "iota": iota,
                "zer": zer,
            }
        )
    res1 = _run(nc, in_maps1)

    # ---- host halo exchange --------------------------------------------
    z1_raw = np.concatenate(
        [
            res1[c]["zt"].transpose(0, 2, 1).reshape(PADR, D)[:RPC]
            for c in range(NCORES)
        ]
    )
    z1_scaled = z1_raw * deg_inv[:, None]

    # ---- launch 2: z2 = M (D^-1 z1_raw) ; out = base + n(y2)/3 ----------
    g2 = _gather_stream(z1_scaled, g_src)
    in_maps2 = []
    for c in range(NCORES):
        in_maps2.append(
            {
                "g": g2[c],
                "rel": rel_bp[c],
                "base": np.ascontiguousarray(res1[c]["out"]),
                "wt": wt2,
                "iota": iota,
                "zer": zer,
            }
        )
    res2 = _run(nc, in_maps2)

    out = np.concatenate(
        [res2[c]["out"][:RPC] for c in range(NCORES)]
    )
    return out.astype(np.float32)


# revision 5
# speedup vs baseline: 1.7154x; 1.7154x over previous
"""Trainium2 Bass kernel for a 2-layer LightGCN-style ItemConv.

Math restructure (W right-multiplication commutes with the linear SpMM):
    y1 = (A x) W1^T,  y2 = (A A x) W1^T W2^T,  A = D^-1 (adj + I)
    out = x/3 + n(y1)/3 + n(y2)/3        (n = row L2-normalize)
Row-normalize kills any positive per-row scaling, so all D^-1 factors are
either folded into the host-side gather stream or cancelled.

Device work per layer (per core, 6250 dst rows):
    z_raw^T tile [100,128] = sum over edge-chunks of  G_chunk^T @ S_chunk
where G_chunk = 128 gathered source rows (host pre-gathers into a
sequential stream, uniform chunk layout across all 8 cores so one SPMD
program serves every core and both layers), and S_chunk is a 0/1 one-hot
[128 edges, 32 dst-window] built on-device with a single DVE is_equal.
Then y_raw = z_raw @ Wt on PE, row-normalized and accumulated.

Two launches of the SAME compiled program; the host performs the halo
exchange (re-gather of layer-1 output) between them.
"""

import sys

sys.path.insert(0, "/opt/trn_rl_repo")

import numpy as np

N = 50000
D = 100
NCORES = 8
RPC = N // NCORES            # 6250 dst rows per core
TILES = (RPC + 127) // 128   # 49
PADR = TILES * 128           # 6272
WIN = 32                     # one-hot width / dst window
NWIN = 128 // WIN            # 4
CE = 128                     # edges per chunk
GCH = 32                     # chunks per staged group

_CACHE: dict = {}
LAST_EXEC_NS: list = []      # appended per launch when TRACE
TRACE = False


def _prep_graph(src, dst):
    """Uniform (across cores) chunk layout for the edge list + self loops."""
    loop = np.arange(N, dtype=np.int64)
    src_all = np.concatenate([np.asarray(src, dtype=np.int64), loop])
    dst_all = np.concatenate([np.asarray(dst, dtype=np.int64), loop])
    deg = np.bincount(dst_all, minlength=N).astype(np.float32)
    deg_inv = (1.0 / deg).astype(np.float32)

    order = np.argsort(dst_all, kind="stable")
    s_s = src_all[order]
    d_s = dst_all[order]

    core = d_s // RPC
    dloc = d_s - core * RPC
    tile = dloc // 128
    wloc = (dloc % 128) // WIN
    slot = (core * TILES + tile) * NWIN + wloc      # globally non-decreasing
    nslots = NCORES * TILES * NWIN

    cnt = np.bincount(slot, minlength=nslots).reshape(NCORES, TILES, NWIN)
    nch = -(-cnt // CE)                              # ceil
    nch = nch.max(axis=0)                            # [TILES, NWIN] uniform
    ncht_raw = int(nch.sum())
    ncht = -(-ncht_raw // GCH) * GCH                 # pad to group multiple

    # chunk offset of each (tile, window) slot in the global chunk stream
    off = np.zeros(TILES * NWIN, dtype=np.int64)
    np.cumsum(nch.reshape(-1)[:-1], out=off[1:])
    off = off.reshape(TILES, NWIN)

    # position of each edge inside its slot (edges already sorted by slot)
    first = np.searchsorted(slot, np.arange(nslots), side="left")
    pos_in_slot = np.arange(slot.size, dtype=np.int64) - first[slot]

    g_src = np.zeros((NCORES, ncht * CE), dtype=np.int64)
    rel = np.full((NCORES, ncht * CE), -1.0, dtype=np.float32)
    gpos = off[tile, wloc] * CE + pos_in_slot        # position inside core stream
    for c in range(NCORES):
        m = core == c
        g_src[c, gpos[m]] = s_s[m]
        rel[c, gpos[m]] = (dloc[m] % WIN).astype(np.float32)

    import ml_dtypes

    rel_bp = np.ascontiguousarray(
        rel.reshape(NCORES, ncht, CE).transpose(0, 2, 1)
    ).astype(ml_dtypes.bfloat16)                     # [NCORES, 128, ncht]
    return g_src, rel_bp, nch, ncht, deg_inv


def _gather_stream(mat, g_src):
    """mat [N, D] -> per-core partition-major bf16 chunk stream [128, ncht, D]."""
    import ml_dtypes

    out = []
    for c in range(NCORES):
        rows = mat[g_src[c]]                         # [ncht*128, D]
        ncht = rows.shape[0] // CE
        out.append(
            np.ascontiguousarray(
                rows.reshape(ncht, CE, D).transpose(1, 0, 2)
            ).astype(ml_dtypes.bfloat16)
        )
    return out


def _build_program(nch, ncht):
    import concourse.bacc as bacc
    import concourse.mybir as mybir
    from concourse import tile

    dt = mybir.dt
    f32 = dt.float32
    bf16 = dt.bfloat16
    nc = bacc.Bacc(
        "TRN2", target_bir_lowering=False, debug=False, num_devices=NCORES
    )

    g_d = nc.dram_tensor("g", [128, ncht, D], bf16, kind="ExternalInput")
    rel_d = nc.dram_tensor("rel", [128, ncht], bf16, kind="ExternalInput")
    base_d = nc.dram_tensor("base", [PADR, D], f32, kind="ExternalInput")
    wt_d = nc.dram_tensor("wt", [D, D], bf16, kind="ExternalInput")
    iota_d = nc.dram_tensor("iota", [128, GCH, WIN], bf16, kind="ExternalInput")
    zer_d = nc.dram_tensor("zer", [128, 128], bf16, kind="ExternalInput")
    zt_d = nc.dram_tensor("zt", [TILES, D, 128], bf16, kind="ExternalOutput")
    out_d = nc.dram_tensor("out", [PADR, D], f32, kind="ExternalOutput")

    ngroups = ncht // GCH

    with tile.TileContext(nc) as tc:
        with (
            tc.tile_pool(name="const", bufs=1) as cp,
            tc.tile_pool(name="gpool", bufs=3) as gp,
            tc.tile_pool(name="spool", bufs=3) as sp,
            tc.tile_pool(name="work", bufs=3) as wp,
            tc.tile_pool(name="small", bufs=4) as smp,
            tc.tile_pool(name="psz", bufs=2, space="PSUM") as pszp,
            tc.tile_pool(name="psy", bufs=2, space="PSUM") as psyp,
        ):
            wt_sb = cp.tile([D, D], bf16)
            nc.sync.dma_start(wt_sb[:], wt_d[:])
            iota_sb = cp.tile([128, GCH, WIN], bf16)
            nc.sync.dma_start(iota_sb[:], iota_d[:])
            zer_sb = cp.tile([128, 128], bf16)
            nc.sync.dma_start(zer_sb[:], zer_d[:])
            rel_sb = cp.tile([128, ncht], bf16)
            nc.sync.dma_start(rel_sb[:], rel_d[:])

            state = {"grp": -1, "g": None, "s": None}

            def ensure_group(cg):
                grp = cg // GCH
                if grp == state["grp"]:
                    return
                state["grp"] = grp
                gt = gp.tile([128, GCH, D], bf16)
                nc.sync.dma_start(
                    gt[:], g_d[:, grp * GCH : (grp + 1) * GCH, :]
                )
                st = sp.tile([128, GCH, WIN], bf16)
                relb = (
                    rel_sb[:, grp * GCH : (grp + 1) * GCH]
                    .unsqueeze(2)
                    .broadcast_to([128, GCH, WIN])
                )
                nc.vector.tensor_tensor(
                    st[:], iota_sb[:], relb, mybir.AluOpType.is_equal
                )
                state["g"] = gt
                state["s"] = st

            cg = 0
            for t in range(TILES):
                pz = pszp.tile([D, 128], f32)
                nc.tensor.matmul(
                    pz[:, :],
                    zer_sb[:, 0:D],
                    zer_sb[:, :],
                    start=True,
                    stop=False,
                )
                ntile = int(nch[t].sum())
                done = 0
                for w in range(NWIN):
                    for _k in range(int(nch[t, w])):
                        ensure_group(cg)
                        ci = cg % GCH
                        done += 1
                        nc.tensor.matmul(
                            pz[:, w * WIN : (w + 1) * WIN],
                            state["g"][:, ci, :],
                            state["s"][:, ci, :],
                            start=False,
                            stop=(done == ntile),
                        )
                        cg += 1

                zt_sb = wp.tile([D, 128], bf16)
                nc.vector.tensor_copy(zt_sb[:], pz[:])
                nc.sync.dma_start(zt_d[t], zt_sb[:])

                py = psyp.tile([128, D], f32)
                nc.tensor.matmul(
                    py[:], zt_sb[:], wt_sb[:], start=True, stop=True
                )

                sqs = wp.tile([128, D], f32)
                ssq = smp.tile([128, 1], f32)
                nc.scalar.activation(
                    sqs[:],
                    py[:],
                    mybir.ActivationFunctionType.Square,
                    accum_out=ssq[:],
                )
                nrm = smp.tile([128, 1], f32)
                nc.scalar.activation(
                    nrm[:],
                    ssq[:],
                    mybir.ActivationFunctionType.Sqrt,
                    scale=9.0,
                )
                rsq = smp.tile([128, 1], f32)
                nc.vector.reciprocal(rsq[:], nrm[:])
                bt = wp.tile([128, D], f32)
                nc.sync.dma_start(bt[:], base_d[t * 128 : (t + 1) * 128, :])
                yn = wp.tile([128, D], f32)
                nc.vector.tensor_scalar(
                    yn[:], py[:], rsq[:], None, mybir.AluOpType.mult
                )
                ot = wp.tile([128, D], f32)
                nc.vector.tensor_tensor(
                    ot[:], yn[:], bt[:], mybir.AluOpType.add
                )
                nc.sync.dma_start(out_d[t * 128 : (t + 1) * 128, :], ot[:])

    nc.compile()
    return nc


def _run(nc, in_maps):
    from concourse.bass_utils import run_bass_kernel_spmd

    res = run_bass_kernel_spmd(
        nc, in_maps, list(range(NCORES)), trace=TRACE
    )
    if res.exec_time_ns is not None:
        LAST_EXEC_NS.append(res.exec_time_ns)
    return res.results


def kernel(features, W, src, dst):
    features = np.asarray(features, dtype=np.float32)
    W = np.asarray(W, dtype=np.float32)

    key = (hash(np.asarray(src).tobytes()), hash(np.asarray(dst).tobytes()))
    if key in _CACHE:
        nc, g_src, rel_bp, nch, ncht, deg_inv = _CACHE[key]
    else:
        g_src, rel_bp, nch, ncht, deg_inv = _prep_graph(src, dst)
        nc = _build_program(nch, ncht)
        _CACHE.clear()
        _CACHE[key] = (nc, g_src, rel_bp, nch, ncht, deg_inv)

    import ml_dtypes

    bf = ml_dtypes.bfloat16
    iota = np.ascontiguousarray(
        np.broadcast_to(np.arange(WIN, dtype=bf), (128, GCH, WIN))
    )
    zer = np.zeros((128, 128), dtype=bf)

    x3 = features / 3.0
    wt1 = np.ascontiguousarray(W[0].T).astype(bf)
    wt2 = np.ascontiguousarray((W[1] @ W[0]).T).astype(bf)

    # ---- launch 1: z1_raw = M x3 ; out = x3 + n(y1)/3 -------------------
    g1 = _gather_stream(x3, g_src)
    in_maps1 = []
    for c in range(NCORES):
        base = np.zeros((PADR, D), dtype=np.float32)
        base[:RPC] = x3[c * RPC : (c + 1) * RPC]
        in_maps1.append(
            {
                "g": g1[c],
                "rel": rel_bp[c],
                "base": base,
                "wt": wt1,
                "iota": iota,
                "zer": zer,
            }
        )
    res1 = _run(nc, in_maps1)

    # ---- host halo exchange --------------------------------------------
    z1_raw = np.concatenate(
        [
            res1[c]["zt"].astype(np.float32).transpose(0, 2, 1).reshape(PADR, D)[:RPC]
            for c in range(NCORES)
        ]
    )
    z1_scaled = z1_raw * deg_inv[:, None]

    # ---- launch 2: z2 = M (D^-1 z1_raw) ; out = base + n(y2)/3 ----------
    g2 = _gather_stream(z1_scaled, g_src)
    in_maps2 = []
    for c in range(NCORES):
        in_maps2.append(
            {
                "g": g2[c],
                "rel": rel_bp[c],
                "base": np.ascontiguousarray(res1[c]["out"]),
                "wt": wt2,
                "iota": iota,
                "zer": zer,
            }
        )
    res2 = _run(nc, in_maps2)

    out = np.concatenate(
        [res2[c]["out"][:RPC] for c in range(NCORES)]
    )
    return out.astype(np.float32)


# revision 6
# speedup vs baseline: 2.9740x; 1.7336x over previous
"""Trainium2 Bass kernel for a 2-layer LightGCN-style ItemConv.

Math restructure (W right-multiplication commutes with the linear SpMM):
    y1 = (A x) W1^T,  y2 = (A A x) W1^T W2^T,  A = D^-1 (adj + I)
    out = x/3 + n(y1)/3 + n(y2)/3        (n = row L2-normalize)
Row-normalize kills any positive per-row scaling, so all D^-1 factors are
either folded into the host-side gather stream or cancelled.

Device work per layer (per core, 6250 dst rows):
    z_raw^T tile [100,128] = sum over edge-chunks of  G_chunk^T @ S_chunk
where G_chunk = 128 gathered source rows (host pre-gathers into a
sequential stream, uniform chunk layout across all 8 cores so one SPMD
program serves every core and both layers), and S_chunk is a 0/1 one-hot
[128 edges, 32 dst-window] built on-device with a single DVE is_equal.
Then y_raw = z_raw @ Wt on PE, row-normalized and accumulated.

Two launches of the SAME compiled program; the host performs the halo
exchange (re-gather of layer-1 output) between them.
"""

import sys

sys.path.insert(0, "/opt/trn_rl_repo")

import numpy as np

N = 50000
D = 100
NCORES = 8
RPC = N // NCORES            # 6250 dst rows per core
TILES = (RPC + 127) // 128   # 49
PADR = TILES * 128           # 6272
WIN = 32                     # one-hot width / dst window
NWIN = 128 // WIN            # 4
CE = 128                     # edges per chunk
GCH = 32                     # chunks per staged group

_CACHE: dict = {}
LAST_EXEC_NS: list = []      # appended per launch when TRACE
TRACE = False


def _prep_graph(src, dst):
    """Uniform (across cores) chunk layout for the edge list + self loops."""
    loop = np.arange(N, dtype=np.int64)
    src_all = np.concatenate([np.asarray(src, dtype=np.int64), loop])
    dst_all = np.concatenate([np.asarray(dst, dtype=np.int64), loop])
    deg = np.bincount(dst_all, minlength=N).astype(np.float32)
    deg_inv = (1.0 / deg).astype(np.float32)

    order = np.argsort(dst_all, kind="stable")
    s_s = src_all[order]
    d_s = dst_all[order]

    core = d_s // RPC
    dloc = d_s - core * RPC
    tile = dloc // 128
    wloc = (dloc % 128) // WIN
    slot = (core * TILES + tile) * NWIN + wloc      # globally non-decreasing
    nslots = NCORES * TILES * NWIN

    cnt = np.bincount(slot, minlength=nslots).reshape(NCORES, TILES, NWIN)
    nch = -(-cnt // CE)                              # ceil
    nch = nch.max(axis=0)                            # [TILES, NWIN] uniform
    ncht_raw = int(nch.sum())
    ncht = -(-ncht_raw // GCH) * GCH                 # pad to group multiple

    # chunk offset of each (tile, window) slot in the global chunk stream
    off = np.zeros(TILES * NWIN, dtype=np.int64)
    np.cumsum(nch.reshape(-1)[:-1], out=off[1:])
    off = off.reshape(TILES, NWIN)

    # position of each edge inside its slot (edges already sorted by slot)
    first = np.searchsorted(slot, np.arange(nslots), side="left")
    pos_in_slot = np.arange(slot.size, dtype=np.int64) - first[slot]

    g_src = np.zeros((NCORES, ncht * CE), dtype=np.int64)
    rel = np.full((NCORES, ncht * CE), -1.0, dtype=np.float32)
    gpos = off[tile, wloc] * CE + pos_in_slot        # position inside core stream
    for c in range(NCORES):
        m = core == c
        g_src[c, gpos[m]] = s_s[m]
        rel[c, gpos[m]] = (dloc[m] % WIN).astype(np.float32)

    import ml_dtypes

    rel_bp = np.ascontiguousarray(
        rel.reshape(NCORES, ncht, CE).transpose(0, 2, 1)
    ).astype(ml_dtypes.bfloat16)                     # [NCORES, 128, ncht]
    return g_src, rel_bp, nch, ncht, deg_inv


def _gather_stream(mat, g_src):
    """mat [N, D] -> per-core partition-major bf16 chunk stream [128, ncht, D]."""
    import ml_dtypes

    out = []
    for c in range(NCORES):
        rows = mat[g_src[c]]                         # [ncht*128, D]
        ncht = rows.shape[0] // CE
        out.append(
            np.ascontiguousarray(
                rows.reshape(ncht, CE, D).transpose(1, 0, 2)
            ).astype(ml_dtypes.bfloat16)
        )
    return out


def _build_program(nch, ncht):
    import concourse.bacc as bacc
    import concourse.mybir as mybir
    from concourse import tile

    dt = mybir.dt
    f32 = dt.float32
    bf16 = dt.bfloat16
    AF = mybir.ActivationFunctionType
    nc = bacc.Bacc(
        "TRN2", target_bir_lowering=False, debug=False, num_devices=NCORES
    )

    g_d = nc.dram_tensor("g", [128, ncht, D], bf16, kind="ExternalInput")
    rel_d = nc.dram_tensor("rel", [128, ncht], bf16, kind="ExternalInput")
    base_d = nc.dram_tensor("base", [128, TILES, D], f32, kind="ExternalInput")
    wt_d = nc.dram_tensor("wt", [D, D], bf16, kind="ExternalInput")
    iota_d = nc.dram_tensor("iota", [128, GCH, WIN], bf16, kind="ExternalInput")
    zer_d = nc.dram_tensor("zer", [128, 128], bf16, kind="ExternalInput")
    zt_d = nc.dram_tensor("zt", [D, TILES, 128], bf16, kind="ExternalOutput")
    out_d = nc.dram_tensor("out", [128, TILES, D], f32, kind="ExternalOutput")

    with tile.TileContext(nc) as tc:
        with (
            tc.tile_pool(name="const", bufs=1) as cp,
            tc.tile_pool(name="gpool", bufs=4) as gp,
            tc.tile_pool(name="spool", bufs=4) as sp,
            tc.tile_pool(name="small", bufs=8) as smp,
            tc.tile_pool(name="psz", bufs=3, space="PSUM") as pszp,
            tc.tile_pool(name="psy", bufs=4, space="PSUM") as psyp,
        ):
            wt_sb = cp.tile([D, D], bf16)
            nc.scalar.dma_start(wt_sb[:], wt_d[:])
            iota_sb = cp.tile([128, GCH, WIN], bf16)
            nc.scalar.dma_start(iota_sb[:], iota_d[:])
            zer_sb = cp.tile([128, 128], bf16)
            nc.scalar.dma_start(zer_sb[:], zer_d[:])
            rel_sb = cp.tile([128, ncht], bf16)
            nc.scalar.dma_start(rel_sb[:], rel_d[:])
            base_sb = cp.tile([128, TILES, D], f32)
            nc.scalar.dma_start(base_sb[:], base_d[:])
            zt_all = cp.tile([D, TILES, 128], bf16)
            out_all = cp.tile([128, TILES, D], f32)

            state = {"grp": -1, "g": None, "s": None}

            def ensure_group(cg):
                grp = cg // GCH
                if grp == state["grp"]:
                    return
                state["grp"] = grp
                gt = gp.tile([128, GCH, D], bf16)
                nc.sync.dma_start(
                    gt[:], g_d[:, grp * GCH : (grp + 1) * GCH, :]
                )
                st = sp.tile([128, GCH, WIN], bf16)
                relb = (
                    rel_sb[:, grp * GCH : (grp + 1) * GCH]
                    .unsqueeze(2)
                    .broadcast_to([128, GCH, WIN])
                )
                nc.vector.tensor_tensor(
                    st[:], iota_sb[:], relb, mybir.AluOpType.is_equal
                )
                state["g"] = gt
                state["s"] = st

            cg = 0
            for t in range(TILES):
                pz = pszp.tile([D, 128], f32)
                nc.tensor.matmul(
                    pz[:, :],
                    zer_sb[:, 0:D],
                    zer_sb[:, :],
                    start=True,
                    stop=False,
                )
                ntile = int(nch[t].sum())
                done = 0
                for w in range(NWIN):
                    for _k in range(int(nch[t, w])):
                        ensure_group(cg)
                        ci = cg % GCH
                        done += 1
                        nc.tensor.matmul(
                            pz[:, w * WIN : (w + 1) * WIN],
                            state["g"][:, ci, :],
                            state["s"][:, ci, :],
                            start=False,
                            stop=(done == ntile),
                        )
                        cg += 1

                # cast z^T tile to bf16 (ACT) -- doubles as PSUM drain
                nc.scalar.activation(zt_all[:, t, :], pz[:], AF.Identity)

                py = psyp.tile([128, D], f32)
                nc.tensor.matmul(
                    py[:], zt_all[:, t, :], wt_sb[:], start=True, stop=True
                )

                sqs = smp.tile([128, D], f32, tag="sqs")
                ssq = smp.tile([128, 1], f32, tag="ssq")
                nc.scalar.activation(
                    sqs[:], py[:], AF.Square, accum_out=ssq[:]
                )
                nrm = smp.tile([128, 1], f32, tag="nrm")
                nc.scalar.activation(nrm[:], ssq[:], AF.Sqrt, scale=9.0)
                rsq = smp.tile([128, 1], f32, tag="rsq")
                nc.vector.reciprocal(rsq[:], nrm[:])
                nc.vector.scalar_tensor_tensor(
                    out_all[:, t, :],
                    py[:],
                    rsq[:],
                    base_sb[:, t, :],
                    mybir.AluOpType.mult,
                    mybir.AluOpType.add,
                )

            nc.gpsimd.dma_start(zt_d[:], zt_all[:])
            nc.sync.dma_start(out_d[:], out_all[:])

    nc.compile()
    return nc


def _run(nc, in_maps):
    from concourse.bass_utils import run_bass_kernel_spmd

    res = run_bass_kernel_spmd(
        nc, in_maps, list(range(NCORES)), trace=TRACE
    )
    if res.exec_time_ns is not None:
        LAST_EXEC_NS.append(res.exec_time_ns)
    return res.results


def kernel(features, W, src, dst):
    features = np.asarray(features, dtype=np.float32)
    W = np.asarray(W, dtype=np.float32)

    key = (hash(np.asarray(src).tobytes()), hash(np.asarray(dst).tobytes()))
    if key in _CACHE:
        nc, g_src, rel_bp, nch, ncht, deg_inv = _CACHE[key]
    else:
        g_src, rel_bp, nch, ncht, deg_inv = _prep_graph(src, dst)
        nc = _build_program(nch, ncht)
        _CACHE.clear()
        _CACHE[key] = (nc, g_src, rel_bp, nch, ncht, deg_inv)

    import ml_dtypes

    bf = ml_dtypes.bfloat16
    iota = np.ascontiguousarray(
        np.broadcast_to(np.arange(WIN, dtype=bf), (128, GCH, WIN))
    )
    zer = np.zeros((128, 128), dtype=bf)

    x3 = features / 3.0
    wt1 = np.ascontiguousarray(W[0].T).astype(bf)
    wt2 = np.ascontiguousarray((W[1] @ W[0]).T).astype(bf)

    # ---- launch 1: z1_raw = M x3 ; out = x3 + n(y1)/3 -------------------
    g1 = _gather_stream(x3, g_src)
    in_maps1 = []
    for c in range(NCORES):
        base = np.zeros((PADR, D), dtype=np.float32)
        base[:RPC] = x3[c * RPC : (c + 1) * RPC]
        base = np.ascontiguousarray(
            base.reshape(TILES, 128, D).transpose(1, 0, 2)
        )
        in_maps1.append(
            {
                "g": g1[c],
                "rel": rel_bp[c],
                "base": base,
                "wt": wt1,
                "iota": iota,
                "zer": zer,
            }
        )
    res1 = _run(nc, in_maps1)

    # ---- host halo exchange --------------------------------------------
    z1_raw = np.concatenate(
        [
            res1[c]["zt"].astype(np.float32).transpose(1, 2, 0).reshape(PADR, D)[:RPC]
            for c in range(NCORES)
        ]
    )
    z1_scaled = z1_raw * deg_inv[:, None]

    # ---- launch 2: z2 = M (D^-1 z1_raw) ; out = base + n(y2)/3 ----------
    g2 = _gather_stream(z1_scaled, g_src)
    in_maps2 = []
    for c in range(NCORES):
        in_maps2.append(
            {
                "g": g2[c],
                "rel": rel_bp[c],
                "base": np.ascontiguousarray(res1[c]["out"]),  # already [128,T,D]
                "wt": wt2,
                "iota": iota,
                "zer": zer,
            }
        )
    res2 = _run(nc, in_maps2)

    out = np.concatenate(
        [
            res2[c]["out"].transpose(1, 0, 2).reshape(PADR, D)[:RPC]
            for c in range(NCORES)
        ]
    )
    return out.astype(np.float32)


# revision 7
# speedup vs baseline: 2.9866x; 1.0043x over previous
"""Trainium2 Bass kernel for a 2-layer LightGCN-style ItemConv.

Math restructure (W right-multiplication commutes with the linear SpMM):
    y1 = (A x) W1^T,  y2 = (A A x) W1^T W2^T,  A = D^-1 (adj + I)
    out = x/3 + n(y1)/3 + n(y2)/3        (n = row L2-normalize)
Row-normalize kills any positive per-row scaling, so all D^-1 factors are
either folded into the host-side gather stream or cancelled.

Device work per layer (per core, 6250 dst rows):
    z_raw^T tile [100,128] = sum over edge-chunks of  G_chunk^T @ S_chunk
where G_chunk = 128 gathered source rows (host pre-gathers into a
sequential stream, uniform chunk layout across all 8 cores so one SPMD
program serves every core and both layers), and S_chunk is a 0/1 one-hot
[128 edges, 32 dst-window] built on-device with a single DVE is_equal.
Then y_raw = z_raw @ Wt on PE, row-normalized and accumulated.

Two launches of the SAME compiled program; the host performs the halo
exchange (re-gather of layer-1 output) between them.
"""

import sys

sys.path.insert(0, "/opt/trn_rl_repo")

import numpy as np

N = 50000
D = 100
NCORES = 8
RPC = N // NCORES            # 6250 dst rows per core
TILES = (RPC + 127) // 128   # 49
PADR = TILES * 128           # 6272
WIN = 32                     # one-hot width / dst window
NWIN = 128 // WIN            # 4
CE = 128                     # edges per chunk
GCH = 32                     # chunks per staged group

_CACHE: dict = {}
LAST_EXEC_NS: list = []      # appended per launch when TRACE
TRACE = False


def _prep_graph(src, dst):
    """Uniform (across cores) chunk layout for the edge list + self loops.

    dst nodes are bin-packed into 32-node windows (a free permutation of
    each core's row space) so window edge-sums land just under chunk
    multiples -- this cuts ceil-padding of the gathered edge stream from
    ~20% to a few percent. node_of[c, v] maps virtual padded row -> node.
    """
    loop = np.arange(N, dtype=np.int64)
    src_all = np.concatenate([np.asarray(src, dtype=np.int64), loop])
    dst_all = np.concatenate([np.asarray(dst, dtype=np.int64), loop])
    deg = np.bincount(dst_all, minlength=N).astype(np.int64)
    deg_inv = (1.0 / deg.astype(np.float32)).astype(np.float32)

    nwindows = TILES * NWIN                          # 196
    # capacity targets (chunks) per window -- identical for every core so
    # the max8 below stays tight
    max_edges = max(
        int(deg[c * RPC : (c + 1) * RPC].sum()) for c in range(NCORES)
    )
    tgt = np.full(nwindows, 4, dtype=np.int64)
    extra = max(0, -(-(max_edges + 1024 - int(tgt.sum()) * CE) // CE))
    tgt[:extra] += 1

    # per-core greedy packing: nodes in degree-desc order into the window
    # with most remaining capacity that still has a free node slot
    node_of = np.full((NCORES, PADR), -1, dtype=np.int64)
    vrow = np.zeros(N, dtype=np.int64)               # node -> virtual row
    for c in range(NCORES):
        nodes = np.arange(c * RPC, (c + 1) * RPC)
        order = np.argsort(-deg[nodes], kind="stable")
        nodes = nodes[order]
        degs = deg[nodes]
        cap = tgt * CE
        slots = np.full(nwindows, 32, dtype=np.int64)
        fill = np.zeros(nwindows, dtype=np.int64)
        for nd, dg in zip(nodes, degs):
            avail = slots > 0
            capm = np.where(avail, cap, -(1 << 40))
            wsel = int(np.argmax(capm))
            cap[wsel] -= dg
            slots[wsel] -= 1
            v = wsel * 32 + (32 - 1 - int(slots[wsel]))
            node_of[c, v] = nd
            vrow[nd] = v
            fill[wsel] += dg

    # edge -> (core, virtual row) -> slot
    core = dst_all // RPC
    v = vrow[dst_all]
    tile = v // 128
    wloc = (v % 128) // WIN
    slot = (core * TILES + tile) * NWIN + wloc
    nslots = NCORES * TILES * NWIN

    eorder = np.argsort(slot, kind="stable")
    s_s = src_all[eorder]
    slot_s = slot[eorder]
    v_s = v[eorder]
    core_s = core[eorder]

    cnt = np.bincount(slot_s, minlength=nslots).reshape(NCORES, TILES, NWIN)
    nch = (-(-cnt // CE)).max(axis=0)                # [TILES, NWIN] uniform
    nch = np.maximum(nch, 1)                         # keep start=True cover
    ncht_raw = int(nch.sum())
    ncht = -(-ncht_raw // GCH) * GCH

    off = np.zeros(TILES * NWIN, dtype=np.int64)
    np.cumsum(nch.reshape(-1)[:-1], out=off[1:])
    off = off.reshape(TILES, NWIN)

    first = np.searchsorted(slot_s, np.arange(nslots), side="left")
    pos_in_slot = np.arange(slot_s.size, dtype=np.int64) - first[slot_s]

    g_src = np.zeros((NCORES, ncht * CE), dtype=np.int64)
    rel = np.full((NCORES, ncht * CE), -1.0, dtype=np.float32)
    gpos = off[tile[eorder], wloc[eorder]] * CE + pos_in_slot
    for c in range(NCORES):
        m = core_s == c
        g_src[c, gpos[m]] = s_s[m]
        rel[c, gpos[m]] = (v_s[m] % WIN).astype(np.float32)

    import ml_dtypes

    rel_bp = np.ascontiguousarray(
        rel.reshape(NCORES, ncht, CE).transpose(0, 2, 1)
    ).astype(ml_dtypes.bfloat16)                     # [NCORES, 128, ncht]
    return g_src, rel_bp, nch, ncht, deg_inv, node_of


def _gather_stream(mat, g_src):
    """mat [N, D] -> per-core partition-major bf16 chunk stream [128, ncht, D]."""
    import ml_dtypes

    out = []
    for c in range(NCORES):
        rows = mat[g_src[c]]                         # [ncht*128, D]
        ncht = rows.shape[0] // CE
        out.append(
            np.ascontiguousarray(
                rows.reshape(ncht, CE, D).transpose(1, 0, 2)
            ).astype(ml_dtypes.bfloat16)
        )
    return out


def _build_program(nch, ncht):
    import concourse.bacc as bacc
    import concourse.mybir as mybir
    from concourse import tile

    dt = mybir.dt
    f32 = dt.float32
    bf16 = dt.bfloat16
    AF = mybir.ActivationFunctionType
    nc = bacc.Bacc(
        "TRN2", target_bir_lowering=False, debug=False, num_devices=NCORES
    )

    g_d = nc.dram_tensor("g", [128, ncht, D], bf16, kind="ExternalInput")
    rel_d = nc.dram_tensor("rel", [128, ncht], bf16, kind="ExternalInput")
    base_d = nc.dram_tensor("base", [128, TILES, D], f32, kind="ExternalInput")
    wt_d = nc.dram_tensor("wt", [D, D], bf16, kind="ExternalInput")
    iota_d = nc.dram_tensor("iota", [128, GCH, WIN], bf16, kind="ExternalInput")
    zer_d = nc.dram_tensor("zer", [128, 128], bf16, kind="ExternalInput")
    zt_d = nc.dram_tensor("zt", [D, TILES, 128], bf16, kind="ExternalOutput")
    out_d = nc.dram_tensor("out", [128, TILES, D], f32, kind="ExternalOutput")

    with tile.TileContext(nc) as tc:
        with (
            tc.tile_pool(name="const", bufs=1) as cp,
            tc.tile_pool(name="gpool", bufs=6) as gp,
            tc.tile_pool(name="spool", bufs=6) as sp,
            tc.tile_pool(name="small", bufs=8) as smp,
            tc.tile_pool(name="psz", bufs=3, space="PSUM") as pszp,
            tc.tile_pool(name="psy", bufs=4, space="PSUM") as psyp,
        ):
            wt_sb = cp.tile([D, D], bf16)
            nc.scalar.dma_start(wt_sb[:], wt_d[:])
            iota_sb = cp.tile([128, GCH, WIN], bf16)
            nc.scalar.dma_start(iota_sb[:], iota_d[:])
            zer_sb = cp.tile([128, 128], bf16)
            nc.scalar.dma_start(zer_sb[:], zer_d[:])
            rel_sb = cp.tile([128, ncht], bf16)
            nc.scalar.dma_start(rel_sb[:], rel_d[:])
            base_sb = cp.tile([128, TILES, D], f32)
            nc.scalar.dma_start(base_sb[:], base_d[:])
            zt_all = cp.tile([D, TILES, 128], bf16)
            out_all = cp.tile([128, TILES, D], f32)

            state = {"grp": -1, "g": None, "s": None}

            def ensure_group(cg):
                grp = cg // GCH
                if grp == state["grp"]:
                    return
                state["grp"] = grp
                gt = gp.tile([128, GCH, D], bf16)
                nc.sync.dma_start(
                    gt[:], g_d[:, grp * GCH : (grp + 1) * GCH, :]
                )
                st = sp.tile([128, GCH, WIN], bf16)
                relb = (
                    rel_sb[:, grp * GCH : (grp + 1) * GCH]
                    .unsqueeze(2)
                    .broadcast_to([128, GCH, WIN])
                )
                nc.vector.tensor_tensor(
                    st[:], iota_sb[:], relb, mybir.AluOpType.is_equal
                )
                state["g"] = gt
                state["s"] = st

            cg = 0
            for t in range(TILES):
                pz = pszp.tile([D, 128], f32)
                for w in range(NWIN):
                    kw = int(nch[t, w])
                    for _k in range(kw):
                        ensure_group(cg)
                        ci = cg % GCH
                        nc.tensor.matmul(
                            pz[:, w * WIN : (w + 1) * WIN],
                            state["g"][:, ci, :],
                            state["s"][:, ci, :],
                            start=(_k == 0),
                            stop=(_k == kw - 1),
                        )
                        cg += 1

                # cast z^T tile to bf16 (ACT) -- doubles as PSUM drain
                nc.scalar.activation(zt_all[:, t, :], pz[:], AF.Identity)

                py = psyp.tile([128, D], f32)
                nc.tensor.matmul(
                    py[:], zt_all[:, t, :], wt_sb[:], start=True, stop=True
                )

                sqs = smp.tile([128, D], f32, tag="sqs")
                ssq = smp.tile([128, 1], f32, tag="ssq")
                nc.scalar.activation(
                    sqs[:], py[:], AF.Square, accum_out=ssq[:]
                )
                nrm = smp.tile([128, 1], f32, tag="nrm")
                nc.scalar.activation(nrm[:], ssq[:], AF.Sqrt, scale=9.0)
                rsq = smp.tile([128, 1], f32, tag="rsq")
                nc.vector.reciprocal(rsq[:], nrm[:])
                nc.vector.scalar_tensor_tensor(
                    out_all[:, t, :],
                    py[:],
                    rsq[:],
                    base_sb[:, t, :],
                    mybir.AluOpType.mult,
                    mybir.AluOpType.add,
                )

            nc.gpsimd.dma_start(zt_d[:], zt_all[:])
            nc.sync.dma_start(out_d[:], out_all[:])

    nc.compile()
    return nc


def _run(nc, in_maps):
    from concourse.bass_utils import run_bass_kernel_spmd

    res = run_bass_kernel_spmd(
        nc, in_maps, list(range(NCORES)), trace=TRACE
    )
    if res.exec_time_ns is not None:
        LAST_EXEC_NS.append(res.exec_time_ns)
    return res.results


def kernel(features, W, src, dst):
    features = np.asarray(features, dtype=np.float32)
    W = np.asarray(W, dtype=np.float32)

    key = (hash(np.asarray(src).tobytes()), hash(np.asarray(dst).tobytes()))
    if key in _CACHE:
        nc, g_src, rel_bp, nch, ncht, deg_inv, node_of = _CACHE[key]
    else:
        g_src, rel_bp, nch, ncht, deg_inv, node_of = _prep_graph(src, dst)
        nc = _build_program(nch, ncht)
        _CACHE.clear()
        _CACHE[key] = (nc, g_src, rel_bp, nch, ncht, deg_inv, node_of)

    import ml_dtypes

    bf = ml_dtypes.bfloat16
    iota = np.ascontiguousarray(
        np.broadcast_to(np.arange(WIN, dtype=bf), (128, GCH, WIN))
    )
    zer = np.zeros((128, 128), dtype=bf)

    x3 = features / 3.0
    wt1 = np.ascontiguousarray(W[0].T).astype(bf)
    wt2 = np.ascontiguousarray((W[1] @ W[0]).T).astype(bf)

    # ---- launch 1: z1_raw = M x3 ; out = x3 + n(y1)/3 -------------------
    g1 = _gather_stream(x3, g_src)
    in_maps1 = []
    for c in range(NCORES):
        base = np.zeros((PADR, D), dtype=np.float32)
        valid = node_of[c] >= 0
        base[valid] = x3[node_of[c][valid]]
        base = np.ascontiguousarray(
            base.reshape(TILES, 128, D).transpose(1, 0, 2)
        )
        in_maps1.append(
            {
                "g": g1[c],
                "rel": rel_bp[c],
                "base": base,
                "wt": wt1,
                "iota": iota,
                "zer": zer,
            }
        )
    res1 = _run(nc, in_maps1)

    # ---- host halo exchange --------------------------------------------
    z1_raw = np.empty((N, D), dtype=np.float32)
    for c in range(NCORES):
        zv = res1[c]["zt"].astype(np.float32).transpose(1, 2, 0).reshape(PADR, D)
        valid = node_of[c] >= 0
        z1_raw[node_of[c][valid]] = zv[valid]
    z1_scaled = z1_raw * deg_inv[:, None]

    # ---- launch 2: z2 = M (D^-1 z1_raw) ; out = base + n(y2)/3 ----------
    g2 = _gather_stream(z1_scaled, g_src)
    in_maps2 = []
    for c in range(NCORES):
        in_maps2.append(
            {
                "g": g2[c],
                "rel": rel_bp[c],
                "base": np.ascontiguousarray(res1[c]["out"]),  # already [128,T,D]
                "wt": wt2,
                "iota": iota,
                "zer": zer,
            }
        )
    res2 = _run(nc, in_maps2)

    out = np.empty((N, D), dtype=np.float32)
    for c in range(NCORES):
        ov = res2[c]["out"].transpose(1, 0, 2).reshape(PADR, D)
        valid = node_of[c] >= 0
        out[node_of[c][valid]] = ov[valid]
    return out.astype(np.float32)


# revision 9
# speedup vs baseline: 3.6257x; 1.2140x over previous
"""Trainium2 Bass kernel for a 2-layer LightGCN-style ItemConv.

Math restructure (W right-multiplication commutes with the linear SpMM):
    y1 = (A x) W1^T,  y2 = (A A x) W1^T W2^T,  A = D^-1 (adj + I)
    out = x/3 + n(y1)/3 + n(y2)/3        (n = row L2-normalize)
Row-normalize kills any positive per-row scaling, so all D^-1 factors are
either folded into the host-side gather stream or cancelled.

Device work per layer (per core, 6250 dst rows):
    z_raw^T tile [100,128] = sum over edge-chunks of  G_chunk^T @ S_chunk
where G_chunk = 128 gathered source rows (host pre-gathers into a
sequential stream, uniform chunk layout across all 8 cores so one SPMD
program serves every core and both layers), and S_chunk is a 0/1 one-hot
[128 edges, 32 dst-window] built on-device with a single DVE is_equal.
Then y_raw = z_raw @ Wt on PE, row-normalized and accumulated.

Two launches of the SAME compiled program; the host performs the halo
exchange (re-gather of layer-1 output) between them.
"""

import sys

sys.path.insert(0, "/opt/trn_rl_repo")

import numpy as np

N = 50000
D = 100
NCORES = 8
RPC = N // NCORES            # 6250 dst rows per core
TILES = (RPC + 127) // 128   # 49
PADR = TILES * 128           # 6272
WIN = 32                     # one-hot width / dst window
NWIN = 128 // WIN            # 4
CE = 128                     # edges per chunk
GCH = 32                     # chunks per staged group

_CACHE: dict = {}
LAST_EXEC_NS: list = []      # appended per launch when TRACE
TRACE = False


def _prep_graph(src, dst):
    """Uniform (across cores) chunk layout for the edge list + self loops.

    dst nodes are bin-packed into 32-node windows (a free permutation of
    each core's row space) so window edge-sums land just under chunk
    multiples -- this cuts ceil-padding of the gathered edge stream from
    ~20% to a few percent. node_of[c, v] maps virtual padded row -> node.
    """
    loop = np.arange(N, dtype=np.int64)
    src_all = np.concatenate([np.asarray(src, dtype=np.int64), loop])
    dst_all = np.concatenate([np.asarray(dst, dtype=np.int64), loop])
    deg = np.bincount(dst_all, minlength=N).astype(np.int64)
    deg_inv = (1.0 / deg.astype(np.float32)).astype(np.float32)

    nwindows = TILES * NWIN                          # 196
    # Tiered packing: light windows target <=4 chunks (sum<=512-eps), heavy
    # windows absorb the rest (<=5 chunks). Same heavy-window set for every
    # core so the cross-core max of per-window ceils stays tight.
    NHEAVY = 61
    node_of = np.full((NCORES, PADR), -1, dtype=np.int64)
    vrow = np.zeros(N, dtype=np.int64)
    for c in range(NCORES):
        nodes = np.arange(c * RPC, (c + 1) * RPC)
        order = np.argsort(-deg[nodes], kind="stable")
        nodes = nodes[order]
        degs = np.concatenate([deg[nodes], np.zeros(PADR - RPC, np.int64)])
        nodes = np.concatenate([nodes, np.full(PADR - RPC, -1, np.int64)])
        total = int(degs.sum())
        # heavy pool = top-a + bottom-b of the desc-sorted list, sized so
        # its sum lands near NHEAVY*634
        npool = NHEAVY * 32
        htarget = NHEAVY * 634
        pre = np.concatenate([[0], np.cumsum(degs)])
        suf = np.concatenate([[0], np.cumsum(degs[::-1])])
        best_a, best_err = 0, 1 << 60
        for a in range(0, npool + 1, 8):
            hsum = pre[a] + suf[npool - a]
            err = abs(int(hsum) - htarget)
            if err < best_err:
                best_err, best_a = err, a
        a = best_a
        b = npool - a
        hidx = np.concatenate([np.arange(a), np.arange(PADR - b, PADR)])
        mask = np.zeros(PADR, bool)
        mask[hidx] = True
        lidx = np.nonzero(~mask)[0]

        def snake(idxs, wins):
            nW = len(wins)
            for i, gi in enumerate(idxs):
                r, j = divmod(i, nW)
                if r % 2:
                    j = nW - 1 - j
                wsel = wins[j]
                nd = nodes[gi]
                if nd >= 0:
                    v = wsel * 32 + r
                    node_of[c, v] = nd
                    vrow[nd] = v

        snake(hidx, list(range(NHEAVY)))
        snake(lidx, list(range(NHEAVY, nwindows)))

    # edge -> (core, virtual row) -> slot
    core = dst_all // RPC
    v = vrow[dst_all]
    tile = v // 128
    wloc = (v % 128) // WIN
    slot = (core * TILES + tile) * NWIN + wloc
    nslots = NCORES * TILES * NWIN

    eorder = np.argsort(slot, kind="stable")
    s_s = src_all[eorder]
    slot_s = slot[eorder]
    v_s = v[eorder]
    core_s = core[eorder]

    cnt = np.bincount(slot_s, minlength=nslots).reshape(NCORES, TILES, NWIN)
    nch = (-(-cnt // CE)).max(axis=0)                # [TILES, NWIN] uniform
    nch = np.maximum(nch, 1)                         # keep start=True cover
    ncht_raw = int(nch.sum())
    ncht = -(-ncht_raw // GCH) * GCH

    off = np.zeros(TILES * NWIN, dtype=np.int64)
    np.cumsum(nch.reshape(-1)[:-1], out=off[1:])
    off = off.reshape(TILES, NWIN)

    first = np.searchsorted(slot_s, np.arange(nslots), side="left")
    pos_in_slot = np.arange(slot_s.size, dtype=np.int64) - first[slot_s]

    g_src = np.zeros((NCORES, ncht * CE), dtype=np.int64)
    rel = np.full((NCORES, ncht * CE), -1.0, dtype=np.float32)
    gpos = off[tile[eorder], wloc[eorder]] * CE + pos_in_slot
    for c in range(NCORES):
        m = core_s == c
        g_src[c, gpos[m]] = s_s[m]
        rel[c, gpos[m]] = (v_s[m] % WIN).astype(np.float32)

    import ml_dtypes

    rel_bp = np.ascontiguousarray(
        rel.reshape(NCORES, ncht, CE).transpose(0, 2, 1)
    ).astype(ml_dtypes.bfloat16)                     # [NCORES, 128, ncht]
    return g_src, rel_bp, nch, ncht, deg_inv, node_of


def _gather_stream(mat, g_src):
    """mat [N, D] -> per-core partition-major bf16 chunk stream [128, ncht, D]."""
    import ml_dtypes

    out = []
    for c in range(NCORES):
        rows = mat[g_src[c]]                         # [ncht*128, D]
        ncht = rows.shape[0] // CE
        out.append(
            np.ascontiguousarray(
                rows.reshape(ncht, CE, D).transpose(1, 0, 2)
            ).astype(ml_dtypes.bfloat16)
        )
    return out


def _build_program(nch, ncht):
    import concourse.bacc as bacc
    import concourse.mybir as mybir
    from concourse import tile

    dt = mybir.dt
    f32 = dt.float32
    bf16 = dt.bfloat16
    AF = mybir.ActivationFunctionType
    nc = bacc.Bacc(
        "TRN2", target_bir_lowering=False, debug=False, num_devices=NCORES
    )

    g_d = nc.dram_tensor("g", [128, ncht, D], bf16, kind="ExternalInput")
    rel_d = nc.dram_tensor("rel", [128, ncht], bf16, kind="ExternalInput")
    base_d = nc.dram_tensor("base", [128, TILES, D], f32, kind="ExternalInput")
    wt_d = nc.dram_tensor("wt", [D, D], bf16, kind="ExternalInput")
    iota_d = nc.dram_tensor("iota", [128, GCH, WIN], bf16, kind="ExternalInput")
    zer_d = nc.dram_tensor("zer", [128, 128], bf16, kind="ExternalInput")
    zt_d = nc.dram_tensor("zt", [D, TILES, 128], bf16, kind="ExternalOutput")
    out_d = nc.dram_tensor("out", [128, TILES, D], f32, kind="ExternalOutput")

    with tile.TileContext(nc) as tc:
        with (
            tc.tile_pool(name="const", bufs=1) as cp,
            tc.tile_pool(name="gpool", bufs=6) as gp,
            tc.tile_pool(name="spool", bufs=6) as sp,
            tc.tile_pool(name="small", bufs=8) as smp,
            tc.tile_pool(name="psz", bufs=3, space="PSUM") as pszp,
            tc.tile_pool(name="psy", bufs=4, space="PSUM") as psyp,
        ):
            wt_sb = cp.tile([D, D], bf16)
            nc.scalar.dma_start(wt_sb[:], wt_d[:])
            iota_sb = cp.tile([128, GCH, WIN], bf16)
            nc.scalar.dma_start(iota_sb[:], iota_d[:])
            zer_sb = cp.tile([128, 128], bf16)
            nc.scalar.dma_start(zer_sb[:], zer_d[:])
            rel_sb = cp.tile([128, ncht], bf16)
            nc.scalar.dma_start(rel_sb[:], rel_d[:])
            base_sb = cp.tile([128, TILES, D], f32)
            zt_all = cp.tile([D, TILES, 128], bf16)
            out_all = cp.tile([128, TILES, D], f32)

            state = {"grp": -1, "g": None, "s": None}

            def ensure_group(cg):
                grp = cg // GCH
                if grp == state["grp"]:
                    return
                state["grp"] = grp
                gt = gp.tile([128, GCH, D], bf16)
                nc.sync.dma_start(
                    gt[:], g_d[:, grp * GCH : (grp + 1) * GCH, :]
                )
                st = sp.tile([128, GCH, WIN], bf16)
                relb = (
                    rel_sb[:, grp * GCH : (grp + 1) * GCH]
                    .unsqueeze(2)
                    .broadcast_to([128, GCH, WIN])
                )
                nc.vector.tensor_tensor(
                    st[:], iota_sb[:], relb, mybir.AluOpType.is_equal
                )
                state["g"] = gt
                state["s"] = st

            SLAB = 7
            cg = 0
            for t in range(TILES):
                if t % SLAB == 0:
                    hi = min(t + SLAB, TILES)
                    nc.scalar.dma_start(
                        base_sb[:, t:hi, :], base_d[:, t:hi, :]
                    )
                pz = pszp.tile([D, 128], f32)
                for w in range(NWIN):
                    kw = int(nch[t, w])
                    for _k in range(kw):
                        ensure_group(cg)
                        ci = cg % GCH
                        nc.tensor.matmul(
                            pz[:, w * WIN : (w + 1) * WIN],
                            state["g"][:, ci, :],
                            state["s"][:, ci, :],
                            start=(_k == 0),
                            stop=(_k == kw - 1),
                        )
                        cg += 1

                # cast z^T tile to bf16 (ACT) -- doubles as PSUM drain
                nc.scalar.activation(zt_all[:, t, :], pz[:], AF.Identity)

                py = psyp.tile([128, D], f32)
                nc.tensor.matmul(
                    py[:], zt_all[:, t, :], wt_sb[:], start=True, stop=True
                )

                sqs = smp.tile([128, D], f32, tag="sqs")
                ssq = smp.tile([128, 1], f32, tag="ssq")
                nc.scalar.activation(
                    sqs[:], py[:], AF.Square, accum_out=ssq[:]
                )
                nrm = smp.tile([128, 1], f32, tag="nrm")
                nc.scalar.activation(nrm[:], ssq[:], AF.Sqrt, scale=9.0)
                rsq = smp.tile([128, 1], f32, tag="rsq")
                nc.vector.reciprocal(rsq[:], nrm[:])
                nc.vector.scalar_tensor_tensor(
                    out_all[:, t, :],
                    py[:],
                    rsq[:],
                    base_sb[:, t, :],
                    mybir.AluOpType.mult,
                    mybir.AluOpType.add,
                )

                if t % SLAB == SLAB - 1 or t == TILES - 1:
                    lo = (t // SLAB) * SLAB
                    nc.gpsimd.dma_start(
                        zt_d[:, lo : t + 1, :], zt_all[:, lo : t + 1, :]
                    )
                    nc.scalar.dma_start(
                        out_d[:, lo : t + 1, :], out_all[:, lo : t + 1, :]
                    )

    nc.compile()
    return nc


def _run(nc, in_maps):
    from concourse.bass_utils import run_bass_kernel_spmd

    res = run_bass_kernel_spmd(
        nc, in_maps, list(range(NCORES)), trace=TRACE
    )
    if res.exec_time_ns is not None:
        LAST_EXEC_NS.append(res.exec_time_ns)
    return res.results


def kernel(features, W, src, dst):
    features = np.asarray(features, dtype=np.float32)
    W = np.asarray(W, dtype=np.float32)

    key = (hash(np.asarray(src).tobytes()), hash(np.asarray(dst).tobytes()))
    if key in _CACHE:
        nc, g_src, rel_bp, nch, ncht, deg_inv, node_of = _CACHE[key]
    else:
        g_src, rel_bp, nch, ncht, deg_inv, node_of = _prep_graph(src, dst)
        nc = _build_program(nch, ncht)
        _CACHE.clear()
        _CACHE[key] = (nc, g_src, rel_bp, nch, ncht, deg_inv, node_of)

    import ml_dtypes

    bf = ml_dtypes.bfloat16
    iota = np.ascontiguousarray(
        np.broadcast_to(np.arange(WIN, dtype=bf), (128, GCH, WIN))
    )
    zer = np.zeros((128, 128), dtype=bf)

    x3 = features / 3.0
    wt1 = np.ascontiguousarray(W[0].T).astype(bf)
    wt2 = np.ascontiguousarray((W[1] @ W[0]).T).astype(bf)

    # ---- launch 1: z1_raw = M x3 ; out = x3 + n(y1)/3 -------------------
    g1 = _gather_stream(x3, g_src)
    in_maps1 = []
    for c in range(NCORES):
        base = np.zeros((PADR, D), dtype=np.float32)
        valid = node_of[c] >= 0
        base[valid] = x3[node_of[c][valid]]
        base = np.ascontiguousarray(
            base.reshape(TILES, 128, D).transpose(1, 0, 2)
        )
        in_maps1.append(
            {
                "g": g1[c],
                "rel": rel_bp[c],
                "base": base,
                "wt": wt1,
                "iota": iota,
                "zer": zer,
            }
        )
    res1 = _run(nc, in_maps1)

    # ---- host halo exchange --------------------------------------------
    z1_raw = np.empty((N, D), dtype=np.float32)
    for c in range(NCORES):
        zv = res1[c]["zt"].astype(np.float32).transpose(1, 2, 0).reshape(PADR, D)
        valid = node_of[c] >= 0
        z1_raw[node_of[c][valid]] = zv[valid]
    z1_scaled = z1_raw * deg_inv[:, None]

    # ---- launch 2: z2 = M (D^-1 z1_raw) ; out = base + n(y2)/3 ----------
    g2 = _gather_stream(z1_scaled, g_src)
    in_maps2 = []
    for c in range(NCORES):
        in_maps2.append(
            {
                "g": g2[c],
                "rel": rel_bp[c],
                "base": np.ascontiguousarray(res1[c]["out"]),  # already [128,T,D]
                "wt": wt2,
                "iota": iota,
                "zer": zer,
            }
        )
    res2 = _run(nc, in_maps2)

    out = np.empty((N, D), dtype=np.float32)
    for c in range(NCORES):
        ov = res2[c]["out"].transpose(1, 0, 2).reshape(PADR, D)
        valid = node_of[c] >= 0
        out[node_of[c][valid]] = ov[valid]
    return out.astype(np.float32)


# revision 10
# speedup vs baseline: 4.0308x; 1.1117x over previous
"""Trainium2 Bass kernel for a 2-layer LightGCN-style ItemConv.

Math restructure (W right-multiplication commutes with the linear SpMM):
    y1 = (A x) W1^T,  y2 = (A A x) W1^T W2^T,  A = D^-1 (adj + I)
    out = x/3 + n(y1)/3 + n(y2)/3        (n = row L2-normalize)
Row-normalize kills any positive per-row scaling, so all D^-1 factors are
either folded into the host-side gather stream or cancelled.

Device work per layer (per core, 6250 dst rows):
    z_raw^T tile [100,128] = sum over edge-chunks of  G_chunk^T @ S_chunk
where G_chunk = 128 gathered source rows (host pre-gathers into a
sequential stream, uniform chunk layout across all 8 cores so one SPMD
program serves every core and both layers), and S_chunk is a 0/1 one-hot
[128 edges, 32 dst-window] built on-device with a single DVE is_equal.
Then y_raw = z_raw @ Wt on PE, row-normalized and accumulated.

Two launches of the SAME compiled program; the host performs the halo
exchange (re-gather of layer-1 output) between them.
"""

import sys

sys.path.insert(0, "/opt/trn_rl_repo")

import numpy as np

N = 50000
D = 100
NCORES = 8
RPC = N // NCORES            # 6250 dst rows per core
TILES = (RPC + 127) // 128   # 49
PADR = TILES * 128           # 6272
WIN = 32                     # one-hot width / dst window
NWIN = 128 // WIN            # 4
CE = 128                     # edges per chunk
GCH = 32                     # chunks per staged group

_CACHE: dict = {}
LAST_EXEC_NS: list = []      # appended per launch when TRACE
TRACE = False


def _prep_graph(src, dst):
    """Uniform (across cores) chunk layout for the edge list + self loops.

    dst nodes are bin-packed into 32-node windows (a free permutation of
    each core's row space) so window edge-sums land just under chunk
    multiples -- this cuts ceil-padding of the gathered edge stream from
    ~20% to a few percent. node_of[c, v] maps virtual padded row -> node.
    """
    loop = np.arange(N, dtype=np.int64)
    src_all = np.concatenate([np.asarray(src, dtype=np.int64), loop])
    dst_all = np.concatenate([np.asarray(dst, dtype=np.int64), loop])
    deg = np.bincount(dst_all, minlength=N).astype(np.int64)
    deg_inv = (1.0 / deg.astype(np.float32)).astype(np.float32)

    nwindows = TILES * NWIN                          # 196
    # Tiered packing: light windows target <=4 chunks (sum<=512-eps), heavy
    # windows absorb the rest (<=5 chunks). Same heavy-window set for every
    # core so the cross-core max of per-window ceils stays tight.
    NHEAVY = 61
    node_of = np.full((NCORES, PADR), -1, dtype=np.int64)
    vrow = np.zeros(N, dtype=np.int64)
    for c in range(NCORES):
        nodes = np.arange(c * RPC, (c + 1) * RPC)
        order = np.argsort(-deg[nodes], kind="stable")
        nodes = nodes[order]
        degs = np.concatenate([deg[nodes], np.zeros(PADR - RPC, np.int64)])
        nodes = np.concatenate([nodes, np.full(PADR - RPC, -1, np.int64)])
        total = int(degs.sum())
        # heavy pool = top-a + bottom-b of the desc-sorted list, sized so
        # its sum lands near NHEAVY*634
        npool = NHEAVY * 32
        htarget = NHEAVY * 634
        pre = np.concatenate([[0], np.cumsum(degs)])
        suf = np.concatenate([[0], np.cumsum(degs[::-1])])
        best_a, best_err = 0, 1 << 60
        for a in range(0, npool + 1, 8):
            hsum = pre[a] + suf[npool - a]
            err = abs(int(hsum) - htarget)
            if err < best_err:
                best_err, best_a = err, a
        a = best_a
        b = npool - a
        hidx = np.concatenate([np.arange(a), np.arange(PADR - b, PADR)])
        mask = np.zeros(PADR, bool)
        mask[hidx] = True
        lidx = np.nonzero(~mask)[0]

        def snake(idxs, wins):
            nW = len(wins)
            for i, gi in enumerate(idxs):
                r, j = divmod(i, nW)
                if r % 2:
                    j = nW - 1 - j
                wsel = wins[j]
                nd = nodes[gi]
                if nd >= 0:
                    v = wsel * 32 + r
                    node_of[c, v] = nd
                    vrow[nd] = v

        snake(hidx, list(range(NHEAVY)))
        snake(lidx, list(range(NHEAVY, nwindows)))

    # edge -> (core, virtual row) -> slot
    core = dst_all // RPC
    v = vrow[dst_all]
    tile = v // 128
    wloc = (v % 128) // WIN
    slot = (core * TILES + tile) * NWIN + wloc
    nslots = NCORES * TILES * NWIN

    eorder = np.argsort(slot, kind="stable")
    s_s = src_all[eorder]
    slot_s = slot[eorder]
    v_s = v[eorder]
    core_s = core[eorder]

    cnt = np.bincount(slot_s, minlength=nslots).reshape(NCORES, TILES, NWIN)
    nch = (-(-cnt // CE)).max(axis=0)                # [TILES, NWIN] uniform
    nch = np.maximum(nch, 1)                         # keep start=True cover
    ncht_raw = int(nch.sum())
    ncht = -(-ncht_raw // GCH) * GCH

    off = np.zeros(TILES * NWIN, dtype=np.int64)
    np.cumsum(nch.reshape(-1)[:-1], out=off[1:])
    off = off.reshape(TILES, NWIN)

    first = np.searchsorted(slot_s, np.arange(nslots), side="left")
    pos_in_slot = np.arange(slot_s.size, dtype=np.int64) - first[slot_s]

    g_src = np.zeros((NCORES, ncht * CE), dtype=np.int64)
    rel = np.full((NCORES, ncht * CE), -1.0, dtype=np.float32)
    gpos = off[tile[eorder], wloc[eorder]] * CE + pos_in_slot
    for c in range(NCORES):
        m = core_s == c
        g_src[c, gpos[m]] = s_s[m]
        rel[c, gpos[m]] = (v_s[m] % WIN).astype(np.float32)

    import ml_dtypes

    rel_bp = np.ascontiguousarray(
        rel.reshape(NCORES, ncht, CE).transpose(0, 2, 1)
    )                                                # [NCORES, 128, ncht] f32
    return g_src, rel_bp, nch, ncht, deg_inv, node_of


def _gather_stream(mat, g_src):
    """mat [N, D] -> per-core partition-major bf16 chunk stream [128, ncht, D]."""
    import ml_dtypes

    out = []
    for c in range(NCORES):
        rows = mat[g_src[c]]                         # [ncht*128, D]
        ncht = rows.shape[0] // CE
        out.append(
            np.ascontiguousarray(
                rows.reshape(ncht, CE, D).transpose(1, 0, 2)
            ).astype(ml_dtypes.bfloat16).reshape(CE, ncht * D)
        )
    return out


def _build_program(nch, ncht):
    import concourse.bacc as bacc
    import concourse.mybir as mybir
    from concourse import tile

    dt = mybir.dt
    f32 = dt.float32
    bf16 = dt.bfloat16
    AF = mybir.ActivationFunctionType
    nc = bacc.Bacc(
        "TRN2", target_bir_lowering=False, debug=False, num_devices=NCORES
    )

    g_d = nc.dram_tensor("g", [128, ncht * D], bf16, kind="ExternalInput")
    rel_d = nc.dram_tensor("rel", [128, ncht], f32, kind="ExternalInput")
    base_d = nc.dram_tensor("base", [128, TILES, D], f32, kind="ExternalInput")
    wt_d = nc.dram_tensor("wt", [D, D], bf16, kind="ExternalInput")
    iota_d = nc.dram_tensor("iota", [128, GCH, WIN], f32, kind="ExternalInput")
    zt_d = nc.dram_tensor("zt", [D, TILES, 128], bf16, kind="ExternalOutput")
    out_d = nc.dram_tensor("out", [128, TILES, D], f32, kind="ExternalOutput")

    with tile.TileContext(nc) as tc:
        with (
            tc.tile_pool(name="const", bufs=1) as cp,
            tc.tile_pool(name="gpool", bufs=6) as gp,
            tc.tile_pool(name="spool", bufs=6) as sp,
            tc.tile_pool(name="small", bufs=8) as smp,
            tc.tile_pool(name="psz", bufs=3, space="PSUM") as pszp,
            tc.tile_pool(name="psy", bufs=4, space="PSUM") as psyp,
        ):
            wt_sb = cp.tile([D, D], bf16)
            nc.scalar.dma_start(wt_sb[:], wt_d[:])
            iota_sb = cp.tile([128, GCH, WIN], f32)
            nc.scalar.dma_start(iota_sb[:], iota_d[:])
            rel_sb = cp.tile([128, ncht], f32)
            nc.scalar.dma_start(rel_sb[:], rel_d[:])
            base_sb = cp.tile([128, TILES, D], f32)
            zt_all = cp.tile([D, TILES, 128], bf16)
            out_all = cp.tile([128, TILES, D], f32)

            state = {"grp": -1, "g": None, "s": None}

            def ensure_group(cg):
                grp = cg // GCH
                if grp == state["grp"]:
                    return
                state["grp"] = grp
                gt = gp.tile([128, GCH * D + 32], bf16)
                nc.sync.dma_start(
                    gt[:, : GCH * D],
                    g_d[:, grp * GCH * D : (grp + 1) * GCH * D],
                )
                st = sp.tile([128, GCH, WIN], bf16)
                relb = (
                    rel_sb[:, grp * GCH : (grp + 1) * GCH]
                    .unsqueeze(2)
                    .broadcast_to([128, GCH, WIN])
                )
                nc.vector.tensor_tensor(
                    st[:], iota_sb[:], relb, mybir.AluOpType.is_equal
                )
                state["g"] = gt
                state["s"] = st

            SLAB = 7
            cg = 0
            for t in range(TILES):
                if t % SLAB == 0:
                    hi = min(t + SLAB, TILES)
                    nc.scalar.dma_start(
                        base_sb[:, t:hi, :], base_d[:, t:hi, :]
                    )
                pz = pszp.tile([128, 128], f32)
                for w in range(NWIN):
                    kw = int(nch[t, w])
                    for _k in range(kw):
                        ensure_group(cg)
                        ci = cg % GCH
                        nc.tensor.matmul(
                            pz[:, w * WIN : (w + 1) * WIN],
                            state["g"][:, ci * D : ci * D + 128],
                            state["s"][:, ci, :],
                            start=(_k == 0),
                            stop=(_k == kw - 1),
                        )
                        cg += 1

                # cast z^T tile to bf16 (ACT) -- doubles as PSUM drain
                nc.scalar.activation(zt_all[:, t, :], pz[0:100, :], AF.Identity)

                py = psyp.tile([128, D], f32)
                nc.tensor.matmul(
                    py[:], zt_all[:, t, :], wt_sb[:], start=True, stop=True
                )

                sqs = smp.tile([128, D], f32, tag="sqs")
                ssq = smp.tile([128, 1], f32, tag="ssq")
                nc.scalar.activation(
                    sqs[:], py[:], AF.Square, accum_out=ssq[:]
                )
                nrm = smp.tile([128, 1], f32, tag="nrm")
                nc.scalar.activation(nrm[:], ssq[:], AF.Sqrt, scale=9.0)
                rsq = smp.tile([128, 1], f32, tag="rsq")
                nc.vector.reciprocal(rsq[:], nrm[:])
                nc.vector.scalar_tensor_tensor(
                    out_all[:, t, :],
                    py[:],
                    rsq[:],
                    base_sb[:, t, :],
                    mybir.AluOpType.mult,
                    mybir.AluOpType.add,
                )

                if t % SLAB == SLAB - 1 or t == TILES - 1:
                    lo = (t // SLAB) * SLAB
                    nc.gpsimd.dma_start(
                        zt_d[:, lo : t + 1, :], zt_all[:, lo : t + 1, :]
                    )
                    nc.scalar.dma_start(
                        out_d[:, lo : t + 1, :], out_all[:, lo : t + 1, :]
                    )

    nc.compile()
    return nc


def _run(nc, in_maps):
    from concourse.bass_utils import run_bass_kernel_spmd

    res = run_bass_kernel_spmd(
        nc, in_maps, list(range(NCORES)), trace=TRACE
    )
    if res.exec_time_ns is not None:
        LAST_EXEC_NS.append(res.exec_time_ns)
    return res.results


def kernel(features, W, src, dst):
    features = np.asarray(features, dtype=np.float32)
    W = np.asarray(W, dtype=np.float32)

    key = (hash(np.asarray(src).tobytes()), hash(np.asarray(dst).tobytes()))
    if key in _CACHE:
        nc, g_src, rel_bp, nch, ncht, deg_inv, node_of = _CACHE[key]
    else:
        g_src, rel_bp, nch, ncht, deg_inv, node_of = _prep_graph(src, dst)
        nc = _build_program(nch, ncht)
        _CACHE.clear()
        _CACHE[key] = (nc, g_src, rel_bp, nch, ncht, deg_inv, node_of)

    iota = np.ascontiguousarray(
        np.broadcast_to(
            np.arange(WIN, dtype=np.float32), (128, GCH, WIN)
        )
    )

    x3 = features / 3.0
    import ml_dtypes

    bf = ml_dtypes.bfloat16
    wt1 = np.ascontiguousarray(W[0].T).astype(bf)
    wt2 = np.ascontiguousarray((W[1] @ W[0]).T).astype(bf)

    # ---- launch 1: z1_raw = M x3 ; out = x3 + n(y1)/3 -------------------
    g1 = _gather_stream(x3, g_src)
    in_maps1 = []
    for c in range(NCORES):
        base = np.zeros((PADR, D), dtype=np.float32)
        valid = node_of[c] >= 0
        base[valid] = x3[node_of[c][valid]]
        base = np.ascontiguousarray(
            base.reshape(TILES, 128, D).transpose(1, 0, 2)
        )
        in_maps1.append(
            {
                "g": g1[c],
                "rel": rel_bp[c],
                "base": base,
                "wt": wt1,
                "iota": iota,
            }
        )
    res1 = _run(nc, in_maps1)

    # ---- host halo exchange --------------------------------------------
    z1_raw = np.empty((N, D), dtype=np.float32)
    for c in range(NCORES):
        zv = res1[c]["zt"].astype(np.float32).transpose(1, 2, 0).reshape(PADR, D)
        valid = node_of[c] >= 0
        z1_raw[node_of[c][valid]] = zv[valid]
    z1_scaled = z1_raw * deg_inv[:, None]

    # ---- launch 2: z2 = M (D^-1 z1_raw) ; out = base + n(y2)/3 ----------
    g2 = _gather_stream(z1_scaled, g_src)
    in_maps2 = []
    for c in range(NCORES):
        in_maps2.append(
            {
                "g": g2[c],
                "rel": rel_bp[c],
                "base": np.ascontiguousarray(res1[c]["out"]),  # already [128,T,D]
                "wt": wt2,
                "iota": iota,
            }
        )
    res2 = _run(nc, in_maps2)

    out = np.empty((N, D), dtype=np.float32)
    for c in range(NCORES):
        ov = res2[c]["out"].transpose(1, 0, 2).reshape(PADR, D)
        valid = node_of[c] >= 0
        out[node_of[c][valid]] = ov[valid]
    return out.astype(np.float32)
